# revision 1
# baseline (speedup 1.0000x reference)
"""Trainium2 Bass kernel for nn_CogAttn: pool -> conv(1,3) -> linear -> relu ->
softmax(axis=1) -> channel-mix einsum.  Data-parallel over batch on 8 cores.

Shapes (hardcoded): x (32, 64, 8, 4000) fp32; conv_w (64,64,1,3); conv_b (64,);
lin_w (64, 624); lin_b (64,).  Output y (32, 64, 8, 4000) fp32.
"""
import sys, json

sys.path.insert(0, "/opt/trn_rl_repo")

import numpy as np
import ml_dtypes

import concourse.bass as bass
import concourse.tile as tile
from concourse import mybir
from concourse.bass_utils import run_bass_kernel_spmd

N_CORES = 8
B, C, H, T, P = 32, 64, 8, 4000, 50
U = T // P              # 80 pooled positions per band
UU = U - 2              # 78 conv-valid positions per band
F = H * UU              # 624
ROW = H * T             # 32000 elements per (batch, channel) row
BPC = B // N_CORES      # 4 batches per core
TILES = BPC // 2        # 2-batch tiles per core
QUARTER = ROW // 4      # 8000 (2 h-bands)
YCHUNK = 512            # matmul free-dim chunk (one PSUM bank)
PS_BANKS = 2            # y psum tile = 2 banks = 1024 cols
YP = YCHUNK * PS_BANKS  # 1024
YSTAGE = 2048           # y sbuf staging cols per out-DMA (1 MiB)

FP32 = mybir.dt.float32
BF16 = mybir.dt.bfloat16


def _split_sync_waits(bir_bytes: bytes, cap: int = 1) -> bytes:
    """walrus in this container only accepts one sync-wait command per
    instruction; spill extra waits onto preceding NoOps on the same engine."""
    m = json.loads(bir_bytes)
    ctr = 0
    for f in m["functions"]:
        for blk in f["blocks"]:
            out = []
            for inst in blk["instructions"]:
                si = inst.get("sync_info")
                ow = (si or {}).get("on_wait") or []
                if len(ow) > cap:
                    spill, keep = ow[: len(ow) - cap], ow[len(ow) - cap:]
                    for i in range(0, len(spill), cap):
                        ctr += 1
                        out.append({
                            "debug": inst.get("debug"),
                            "engine": inst["engine"],
                            "ins": [],
                            "name": f"{inst['name']}-wsplit{ctr}",
                            "opcode": "NoOp",
                            "outs": [],
                            "sync_info": {"on_update": [],
                                          "on_wait": spill[i:i + cap]},
                        })
                    si["on_wait"] = keep
                out.append(inst)
            blk["instructions"] = out
    return json.dumps(m).encode()


def _attn_tail(nc, soft, scps, b2_s):
    """scores psum -> +bias -> relu -> per-block softmax -> block-diag attn."""
    scores = soft.tile([128, 128], FP32, name="scores")
    nc.vector.tensor_add(out=scores, in0=scps, in1=b2_s)
    nc.vector.tensor_scalar_max(out=scores, in0=scores, scalar1=0.0)

    negmax = soft.tile([128, 1], FP32, name="negmax")
    exps = soft.tile([128, 128], FP32, name="exps")
    sums = soft.tile([128, 1], FP32, name="sums")
    recip = soft.tile([128, 1], FP32, name="recip")
    attn = soft.tile([128, 128], BF16, name="attn")
    nc.vector.memset(attn, 0.0)
    for g in range(2):
        rs = slice(64 * g, 64 * (g + 1))
        nc.vector.reduce_max(
            out=negmax[rs], in_=scores[rs, rs],
            axis=mybir.AxisListType.X, negate=True)
        nc.scalar.activation(
            out=exps[rs, rs], in_=scores[rs, rs],
            func=mybir.ActivationFunctionType.Exp,
            bias=negmax[rs], scale=1.0)
        nc.vector.reduce_sum(
            out=sums[rs], in_=exps[rs, rs],
            axis=mybir.AxisListType.X)
    nc.vector.reciprocal(out=recip, in_=sums)
    for g in range(2):
        rs = slice(64 * g, 64 * (g + 1))
        nc.vector.tensor_scalar_mul(
            out=attn[rs, rs], in0=exps[rs, rs], scalar1=recip[rs])
    return attn


def build_nc(tiles=TILES, bench_mode="full"):
    nc = bass.Bass()
    x = nc.dram_tensor("x", (tiles, 128, ROW), FP32, kind="ExternalInput")
    wblk = nc.dram_tensor("wblk", (128, 3, 128), BF16, kind="ExternalInput")
    linwt = nc.dram_tensor("linwt", (UU, H, 128), BF16, kind="ExternalInput")
    b2 = nc.dram_tensor("b2", (128, 128), FP32, kind="ExternalInput")
    y = nc.dram_tensor("y", (tiles, 128, ROW), FP32, kind="ExternalOutput")

    xdt = FP32 if bench_mode == "dma_in_f32" else BF16
    xbufs = 1 if bench_mode == "dma_in_f32" else 2
    with tile.TileContext(nc) as tc:
        with (
            tc.tile_pool(name="consts", bufs=1) as consts,
            tc.tile_pool(name="xin", bufs=xbufs) as xin,
            tc.tile_pool(name="mid", bufs=2) as mid,
            tc.tile_pool(name="soft", bufs=2) as soft,
            tc.tile_pool(name="ystage", bufs=3) as ystage,
            tc.tile_pool(name="cpsum", bufs=2, space="PSUM") as cpsum,
            tc.tile_pool(name="spsum", bufs=2, space="PSUM") as spsum,
            tc.tile_pool(name="ypsum", bufs=2, space="PSUM") as ypsum,
        ):
            wblk_s = consts.tile([128, 3, 128], BF16)
            nc.sync.dma_start(out=wblk_s, in_=wblk[:, :, :])
            linwt_s = consts.tile([UU, H, 128], BF16)
            nc.sync.dma_start(out=linwt_s, in_=linwt[:, :, :])
            b2_s = consts.tile([128, 128], FP32)
            nc.sync.dma_start(out=b2_s, in_=b2[:, :])

            prev_attn_last = None
            for it in range(tiles):
                # --- load x tile (cast fp32 -> bf16 during DMA) + per-quarter
                # pooling / conv / scores so attn is ready right after the load
                xt = xin.tile([128, ROW], xdt, name="xt")
                pooledf = mid.tile([128, H * U], FP32, name="pooledf")
                pooled = mid.tile([128, H, U], BF16, name="pooled")
                reprt = mid.tile([UU, H, 128], BF16, name="reprt")
                do_pool = bench_mode not in ("dma_in", "dma_in_f32", "y_only")
                do_chain = bench_mode in ("full", "no_yout")
                scps = spsum.tile([128, 128], FP32, name="scps") if do_chain else None
                if do_pool and prev_attn_last is not None:
                    # Ordering gadget: these sliver copies read the previous
                    # tile's attn and write into each pooling output region,
                    # so WAW forces this tile's reduces behind the previous
                    # softmax chain in the in-order DVE stream.
                    for q in range(4):
                        nc.vector.tensor_copy(
                            out=pooledf[:, q * 160:q * 160 + 1],
                            in_=prev_attn_last[:, 0:1])
                for q in range(4):
                    sl = slice(q * QUARTER, (q + 1) * QUARTER)
                    nc.gpsimd.dma_start(out=xt[:, sl], in_=x[it, :, sl])
                    if not do_pool:
                        continue
                    nc.vector.reduce_sum(
                        out=pooledf[:, q * (QUARTER // P):(q + 1) * (QUARTER // P)],
                        in_=xt[:, sl].rearrange("p (u w) -> p u w", w=P),
                        axis=mybir.AxisListType.X,
                    )
                    if not do_chain:
                        continue
                    nc.scalar.copy(
                        out=pooled[:, 2 * q:2 * q + 2, :],
                        in_=pooledf[:, q * 160:(q + 1) * 160]
                            .rearrange("p (h u) -> p h u", h=2))
                    for h in (2 * q, 2 * q + 1):
                        cps = cpsum.tile([UU, 128], FP32, name="cps")
                        for tap in range(3):
                            nc.tensor.matmul(
                                out=cps,
                                lhsT=pooled[:, h, tap:tap + UU],
                                rhs=wblk_s[:, tap, :],
                                start=(tap == 0), stop=(tap == 2),
                                skip_group_check=True,
                            )
                        nc.scalar.copy(out=reprt[:, h, :], in_=cps)
                        nc.tensor.matmul(
                            out=scps,
                            lhsT=linwt_s[:, h, :],
                            rhs=reprt[:, h, :],
                            start=(h == 0), stop=(h == H - 1),
                            skip_group_check=True,
                        )
                if not do_pool and bench_mode != "y_only":
                    # minimal consumer so DMAs aren't dead code
                    snk = mid.tile([128, 4], xdt, name="snk")
                    for q in range(4):
                        nc.vector.tensor_copy(
                            out=snk[:, q:q + 1], in_=xt[:, q * QUARTER:q * QUARTER + 1])
                    continue
                if bench_mode == "pool":
                    snk = mid.tile([128, 4], FP32, name="snk")
                    nc.vector.tensor_copy(out=snk[:, 0:1], in_=pooledf[:, 0:1])
                    prev_attn_last = snk
                    continue
                if bench_mode == "y_only":
                    attn = soft.tile([128, 128], BF16, name="attn")
                    nc.vector.memset(attn, 0.0)
                else:
                    attn = _attn_tail(nc, soft, scps, b2_s)
                    prev_attn_last = attn

                # --- y tile = attn^T-blockdiag @ x tile, chunked over columns
                for j0 in range(0, ROW, YSTAGE):
                    stg_w = min(YSTAGE, ROW - j0)
                    yst = ystage.tile([128, YSTAGE], FP32, name="yst")
                    for p0 in range(0, stg_w, YP):
                        pw = min(YP, stg_w - p0)
                        yp = ypsum.tile([128, YP], FP32, name="yp")
                        for m0 in range(0, pw, YCHUNK):
                            mw = min(YCHUNK, pw - m0)
                            nc.tensor.matmul(
                                out=yp[:, m0:m0 + mw],
                                lhsT=attn,
                                rhs=xt[:, j0 + p0 + m0:j0 + p0 + m0 + mw],
                                start=True, stop=True,
                            )
                        nc.scalar.copy(out=yst[:, p0:p0 + pw], in_=yp[:, :pw])
                    if bench_mode != "no_yout":
                        nc.sync.dma_start(out=y[it, :, j0:j0 + stg_w],
                                          in_=yst[:, :stg_w])
                    else:
                        nc.sync.dma_start(out=y[it, :, j0:j0 + 16],
                                          in_=yst[:, :16])

    orig = nc.to_json_bytes
    nc.to_json_bytes = lambda: _split_sync_waits(orig())
    return nc


def prep_params(conv_w, conv_b, lin_w, lin_b):
    conv_w = np.asarray(conv_w, np.float32)
    conv_b = np.asarray(conv_b, np.float32)
    lin_w = np.asarray(lin_w, np.float32)
    lin_b = np.asarray(lin_b, np.float32)

    # moving operand of conv matmul: [i, o] block-diag per tap, pooling 1/P folded
    wblk = np.zeros((3, 128, 128), np.float32)
    for tap in range(3):
        w_io = conv_w[:, :, 0, tap].T / P        # [i, o]
        wblk[tap, :64, :64] = w_io
        wblk[tap, 64:, 64:] = w_io
    wblk = np.ascontiguousarray(wblk.transpose(1, 0, 2)).astype(ml_dtypes.bfloat16)

    # stationary of scores matmul: lin_w^T duplicated to both column halves,
    # laid out [f_in_band, band, o_dup]
    lin_wt = lin_w.T                              # [F, o] = [624, 64]
    lin_dup = np.concatenate([lin_wt, lin_wt], axis=1)   # [624, 128]
    linwt = np.ascontiguousarray(
        lin_dup.reshape(H, UU, 128).transpose(1, 0, 2)).astype(ml_dtypes.bfloat16)

    # combined bias: scores[c, o] needs + lin_b[o] + conv_b[c] * sum_f lin_w[o, f]
    L = lin_w.sum(axis=1)                         # [o]
    Bm = lin_b[:, None] + L[:, None] * conv_b[None, :]    # [o, c]
    b2 = np.tile(Bm, (2, 2)).astype(np.float32)   # [128, 128]
    return wblk, linwt, b2


_NC_CACHE = {}


def kernel(x, conv_w, conv_b, lin_w, lin_b, _want_trace=False):
    x = np.asarray(x, np.float32)
    wblk, linwt, b2 = prep_params(conv_w, conv_b, lin_w, lin_b)

    if "nc" not in _NC_CACHE:
        _NC_CACHE["nc"] = build_nc()
    nc = _NC_CACHE["nc"]

    in_maps = []
    for c in range(N_CORES):
        shard = np.ascontiguousarray(
            x[c * BPC:(c + 1) * BPC].reshape(TILES, 128, ROW))
        in_maps.append({"x": shard, "wblk": wblk, "linwt": linwt, "b2": b2})

    res = run_bass_kernel_spmd(
        nc, in_maps, core_ids=list(range(N_CORES)), trace=_want_trace)

    y = np.concatenate(
        [res.results[c]["y"].reshape(BPC, C, H, T) for c in range(N_CORES)],
        axis=0)
    if _want_trace:
        kernel._last_result = res
    return y



# revision 4
# speedup vs baseline: 1.1579x; 1.1579x over previous
"""Trainium2 Bass kernel for nn_CogAttn: pool -> conv(1,3) -> linear -> relu ->
softmax(axis=1) -> channel-mix einsum.  Data-parallel over batch on 8 cores.

Shapes (hardcoded): x (32, 64, 8, 4000) fp32; conv_w (64,64,1,3); conv_b (64,);
lin_w (64, 624); lin_b (64,).  Output y (32, 64, 8, 4000) fp32.
"""
import sys, json

sys.path.insert(0, "/opt/trn_rl_repo")

import numpy as np
import ml_dtypes

import concourse.bass as bass
import concourse.tile as tile
from concourse import mybir
from concourse.bass_utils import run_bass_kernel_spmd

N_CORES = 8
B, C, H, T, P = 32, 64, 8, 4000, 50
U = T // P              # 80 pooled positions per band
UU = U - 2              # 78 conv-valid positions per band
F = H * UU              # 624
ROW = H * T             # 32000 elements per (batch, channel) row
BPC = B // N_CORES      # 4 batches per core
TILES = BPC // 2        # 2-batch tiles per core
QUARTER = ROW // 4      # 8000 (2 h-bands)
YCHUNK = 512            # matmul free-dim chunk (one PSUM bank)
PS_BANKS = 2            # y psum tile = 2 banks = 1024 cols
YP = YCHUNK * PS_BANKS  # 1024
YSTAGE = 2048           # y sbuf staging cols per out-DMA (1 MiB)

FP32 = mybir.dt.float32
BF16 = mybir.dt.bfloat16


def _split_sync_waits(bir_bytes: bytes, cap: int = 1) -> bytes:
    """walrus in this container only accepts one sync-wait command per
    instruction; spill extra waits onto preceding NoOps on the same engine."""
    m = json.loads(bir_bytes)
    ctr = 0
    for f in m["functions"]:
        for blk in f["blocks"]:
            out = []
            for inst in blk["instructions"]:
                si = inst.get("sync_info")
                ow = (si or {}).get("on_wait") or []
                if len(ow) > cap:
                    spill, keep = ow[: len(ow) - cap], ow[len(ow) - cap:]
                    for i in range(0, len(spill), cap):
                        ctr += 1
                        out.append({
                            "debug": inst.get("debug"),
                            "engine": inst["engine"],
                            "ins": [],
                            "name": f"{inst['name']}-wsplit{ctr}",
                            "opcode": "NoOp",
                            "outs": [],
                            "sync_info": {"on_update": [],
                                          "on_wait": spill[i:i + cap]},
                        })
                    si["on_wait"] = keep
                out.append(inst)
            blk["instructions"] = out
    return json.dumps(m).encode()


def _attn_tail(nc, soft, scps, b2_s):
    """scores psum -> +bias -> relu -> per-block softmax -> block-diag attn."""
    scores = soft.tile([128, 128], FP32, name="scores")
    nc.vector.tensor_add(out=scores, in0=scps, in1=b2_s)
    nc.vector.tensor_scalar_max(out=scores, in0=scores, scalar1=0.0)

    negmax = soft.tile([128, 1], FP32, name="negmax")
    exps = soft.tile([128, 128], FP32, name="exps")
    sums = soft.tile([128, 1], FP32, name="sums")
    recip = soft.tile([128, 1], FP32, name="recip")
    attn = soft.tile([128, 128], BF16, name="attn")
    nc.vector.memset(attn, 0.0)
    for g in range(2):
        rs = slice(64 * g, 64 * (g + 1))
        nc.vector.reduce_max(
            out=negmax[rs], in_=scores[rs, rs],
            axis=mybir.AxisListType.X, negate=True)
        nc.scalar.activation(
            out=exps[rs, rs], in_=scores[rs, rs],
            func=mybir.ActivationFunctionType.Exp,
            bias=negmax[rs], scale=1.0)
        nc.vector.reduce_sum(
            out=sums[rs], in_=exps[rs, rs],
            axis=mybir.AxisListType.X)
    nc.vector.reciprocal(out=recip, in_=sums)
    for g in range(2):
        rs = slice(64 * g, 64 * (g + 1))
        nc.vector.tensor_scalar_mul(
            out=attn[rs, rs], in0=exps[rs, rs], scalar1=recip[rs])
    return attn


def build_nc(tiles=TILES, bench_mode="full"):
    nc = bass.Bass()
    x = nc.dram_tensor("x", (tiles, 128, ROW), FP32, kind="ExternalInput")
    wblk = nc.dram_tensor("wblk", (128, 3, 128), BF16, kind="ExternalInput")
    linwt = nc.dram_tensor("linwt", (UU, H, 128), BF16, kind="ExternalInput")
    b2 = nc.dram_tensor("b2", (128, 128), FP32, kind="ExternalInput")
    y = nc.dram_tensor("y", (tiles, 128, ROW), BF16, kind="ExternalOutput")

    xdt = FP32 if bench_mode == "dma_in_f32" else BF16
    xbufs = 1 if bench_mode == "dma_in_f32" else 2
    with tile.TileContext(nc) as tc:
        with (
            tc.tile_pool(name="consts", bufs=1) as consts,
            tc.tile_pool(name="xin", bufs=xbufs) as xin,
            tc.tile_pool(name="mid", bufs=2) as mid,
            tc.tile_pool(name="soft", bufs=2) as soft,
            tc.tile_pool(name="ystage", bufs=3) as ystage,
            tc.tile_pool(name="cpsum", bufs=2, space="PSUM") as cpsum,
            tc.tile_pool(name="spsum", bufs=2, space="PSUM") as spsum,
            tc.tile_pool(name="ypsum", bufs=2, space="PSUM") as ypsum,
        ):
            wblk_s = consts.tile([128, 3, 128], BF16)
            nc.sync.dma_start(out=wblk_s, in_=wblk[:, :, :])
            linwt_s = consts.tile([UU, H, 128], BF16)
            nc.sync.dma_start(out=linwt_s, in_=linwt[:, :, :])
            b2_s = consts.tile([128, 128], FP32)
            nc.sync.dma_start(out=b2_s, in_=b2[:, :])

            prev_attn_last = None
            for it in range(tiles):
                # --- load x tile (cast fp32 -> bf16 during DMA) + per-quarter
                # pooling / conv / scores so attn is ready right after the load
                xt = xin.tile([128, ROW], xdt, name="xt")
                pooledf = mid.tile([128, H * U], FP32, name="pooledf")
                pooled = mid.tile([128, H, U], BF16, name="pooled")
                reprt = mid.tile([UU, H, 128], BF16, name="reprt")
                do_pool = bench_mode not in ("dma_in", "dma_in_f32", "y_only")
                do_chain = bench_mode in ("full", "no_yout")
                scps = spsum.tile([128, 128], FP32, name="scps") if do_chain else None
                if do_pool and prev_attn_last is not None:
                    # Ordering gadget: these sliver copies read the previous
                    # tile's attn and write into each pooling output region,
                    # so WAW forces this tile's reduces behind the previous
                    # softmax chain in the in-order DVE stream.
                    for q in range(4):
                        nc.vector.tensor_copy(
                            out=pooledf[:, q * 160:q * 160 + 1],
                            in_=prev_attn_last[:, 0:1])
                for q in range(4):
                    sl = slice(q * QUARTER, (q + 1) * QUARTER)
                    nc.gpsimd.dma_start(out=xt[:, sl], in_=x[it, :, sl])
                    if not do_pool:
                        continue
                    nc.vector.reduce_sum(
                        out=pooledf[:, q * (QUARTER // P):(q + 1) * (QUARTER // P)],
                        in_=xt[:, sl].rearrange("p (u w) -> p u w", w=P),
                        axis=mybir.AxisListType.X,
                    )
                    if not do_chain:
                        continue
                    nc.scalar.copy(
                        out=pooled[:, 2 * q:2 * q + 2, :],
                        in_=pooledf[:, q * 160:(q + 1) * 160]
                            .rearrange("p (h u) -> p h u", h=2))
                    for h in (2 * q, 2 * q + 1):
                        cps = cpsum.tile([UU, 128], FP32, name="cps")
                        for tap in range(3):
                            nc.tensor.matmul(
                                out=cps,
                                lhsT=pooled[:, h, tap:tap + UU],
                                rhs=wblk_s[:, tap, :],
                                start=(tap == 0), stop=(tap == 2),
                                skip_group_check=True,
                            )
                        nc.scalar.copy(out=reprt[:, h, :], in_=cps)
                        nc.tensor.matmul(
                            out=scps,
                            lhsT=linwt_s[:, h, :],
                            rhs=reprt[:, h, :],
                            start=(h == 0), stop=(h == H - 1),
                            skip_group_check=True,
                        )
                if not do_pool and bench_mode != "y_only":
                    # minimal consumer so DMAs aren't dead code
                    snk = mid.tile([128, 4], xdt, name="snk")
                    for q in range(4):
                        nc.vector.tensor_copy(
                            out=snk[:, q:q + 1], in_=xt[:, q * QUARTER:q * QUARTER + 1])
                    continue
                if bench_mode == "pool":
                    snk = mid.tile([128, 4], FP32, name="snk")
                    nc.vector.tensor_copy(out=snk[:, 0:1], in_=pooledf[:, 0:1])
                    prev_attn_last = snk
                    continue
                if bench_mode == "y_only":
                    attn = soft.tile([128, 128], BF16, name="attn")
                    nc.vector.memset(attn, 0.0)
                else:
                    attn = _attn_tail(nc, soft, scps, b2_s)
                    prev_attn_last = attn

                # --- y tile = attn^T-blockdiag @ x tile, chunked over columns
                for j0 in range(0, ROW, YSTAGE):
                    stg_w = min(YSTAGE, ROW - j0)
                    yst = ystage.tile([128, YSTAGE], BF16, name="yst")
                    for p0 in range(0, stg_w, YP):
                        pw = min(YP, stg_w - p0)
                        yp = ypsum.tile([128, YP], FP32, name="yp")
                        for m0 in range(0, pw, YCHUNK):
                            mw = min(YCHUNK, pw - m0)
                            nc.tensor.matmul(
                                out=yp[:, m0:m0 + mw],
                                lhsT=attn,
                                rhs=xt[:, j0 + p0 + m0:j0 + p0 + m0 + mw],
                                start=True, stop=True,
                            )
                        nc.scalar.copy(out=yst[:, p0:p0 + pw], in_=yp[:, :pw])
                    if bench_mode != "no_yout":
                        nc.sync.dma_start(out=y[it, :, j0:j0 + stg_w],
                                          in_=yst[:, :stg_w])
                    else:
                        nc.sync.dma_start(out=y[it, :, j0:j0 + 16],
                                          in_=yst[:, :16])

    orig = nc.to_json_bytes
    nc.to_json_bytes = lambda: _split_sync_waits(orig())
    return nc


def prep_params(conv_w, conv_b, lin_w, lin_b):
    conv_w = np.asarray(conv_w, np.float32)
    conv_b = np.asarray(conv_b, np.float32)
    lin_w = np.asarray(lin_w, np.float32)
    lin_b = np.asarray(lin_b, np.float32)

    # moving operand of conv matmul: [i, o] block-diag per tap, pooling 1/P folded
    wblk = np.zeros((3, 128, 128), np.float32)
    for tap in range(3):
        w_io = conv_w[:, :, 0, tap].T / P        # [i, o]
        wblk[tap, :64, :64] = w_io
        wblk[tap, 64:, 64:] = w_io
    wblk = np.ascontiguousarray(wblk.transpose(1, 0, 2)).astype(ml_dtypes.bfloat16)

    # stationary of scores matmul: lin_w^T duplicated to both column halves,
    # laid out [f_in_band, band, o_dup]
    lin_wt = lin_w.T                              # [F, o] = [624, 64]
    lin_dup = np.concatenate([lin_wt, lin_wt], axis=1)   # [624, 128]
    linwt = np.ascontiguousarray(
        lin_dup.reshape(H, UU, 128).transpose(1, 0, 2)).astype(ml_dtypes.bfloat16)

    # combined bias: scores[c, o] needs + lin_b[o] + conv_b[c] * sum_f lin_w[o, f]
    L = lin_w.sum(axis=1)                         # [o]
    Bm = lin_b[:, None] + L[:, None] * conv_b[None, :]    # [o, c]
    b2 = np.tile(Bm, (2, 2)).astype(np.float32)   # [128, 128]
    return wblk, linwt, b2


_NC_CACHE = {}


def kernel(x, conv_w, conv_b, lin_w, lin_b, _want_trace=False):
    x = np.asarray(x, np.float32)
    wblk, linwt, b2 = prep_params(conv_w, conv_b, lin_w, lin_b)

    if "nc" not in _NC_CACHE:
        _NC_CACHE["nc"] = build_nc()
    nc = _NC_CACHE["nc"]

    in_maps = []
    for c in range(N_CORES):
        shard = np.ascontiguousarray(
            x[c * BPC:(c + 1) * BPC].reshape(TILES, 128, ROW))
        in_maps.append({"x": shard, "wblk": wblk, "linwt": linwt, "b2": b2})

    res = run_bass_kernel_spmd(
        nc, in_maps, core_ids=list(range(N_CORES)), trace=_want_trace)

    y = np.concatenate(
        [res.results[c]["y"].astype(np.float32).reshape(BPC, C, H, T)
         for c in range(N_CORES)],
        axis=0)
    if _want_trace:
        kernel._last_result = res
    return y

